# revision 1
# baseline (speedup 1.0000x reference)
"""DeepSeek-V3 MLA attention (B=1, S=1024, D=7168, H=128) on 8 Trainium2
NeuronCores.

Sharding: tensor-parallel over the 128 heads (16 heads/core) for
q_b/kv_b/attention; the small latent projections (wq_a, wkv_a) are
M-sharded (256 rows/core) with an AllGather of the rms-normed latents;
k_pe + the rmsnorm sum-of-squares are K-sharded and combined with one
small AllReduce; head outputs are AllGathered (bf16) and o_proj is
row-sharded (896 output features/core) so no output collective is needed.

All activations are kept feature-major [feat, token]; matmuls run in bf16
with fp32 PSUM accumulation; softmax runs without max-subtraction (scores
are O(5) for this problem) with exp in fp32->bf16 and exact causal
masking via a 0/1 triangular mask on the diagonal tiles.
"""

import os
from contextlib import ExitStack

import numpy as np
import ml_dtypes

import concourse.bass as bass
import concourse.mybir as mybir
import concourse.tile as tile
from concourse import bacc
from concourse.bass_utils import run_bass_kernel_spmd
from concourse.masks import make_upper_triangular

bf16 = ml_dtypes.bfloat16
F32 = mybir.dt.float32
BF = mybir.dt.bfloat16

B, S, D = 1, 1024, 7168
H, DN, DR, DV = 128, 128, 64, 128
DQ = DN + DR                  # 192
RQ, RKV = 1536, 512
EPS = 1e-6
SCALE = float(DQ) ** -0.5
NC = 8
HC = H // NC                  # 16 heads per core
T = S
KT_X = D // 128               # 56
KT_Q = RQ // 128              # 12
KT_KV = RKV // 128            # 4
MT_QB = (HC * DQ) // 128      # 24 (16 nope tiles + 8 rope tiles)
DEBUG = bool(int(os.environ.get("BASSMLA_DEBUG", "0")))

_CACHE = {}


def _build():
    nc = bacc.Bacc("TRN2", target_bir_lowering=False, debug=False, num_devices=NC)

    x_in = nc.dram_tensor("x", [128, KT_X, T], BF, kind="ExternalInput").ap()
    xkpe_in = nc.dram_tensor("xkpe", [128, 7, T], BF, kind="ExternalInput").ap()
    wa_in = nc.dram_tensor("wa", [128, KT_X, 2, 128], BF, kind="ExternalInput").ap()
    wkpe_in = nc.dram_tensor("wkpe", [128, 7, DR], BF, kind="ExternalInput").ap()
    ccss_in = nc.dram_tensor("ccss", [128, 2 * T], F32, kind="ExternalInput").ap()
    wqb_in = nc.dram_tensor("wqb", [128, MT_QB, KT_Q, 128], BF, kind="ExternalInput").ap()
    wkn_in = nc.dram_tensor("wkn", [128, HC, KT_KV, 128], BF, kind="ExternalInput").ap()
    wv_in = nc.dram_tensor("wv", [128, KT_KV, HC * DV], BF, kind="ExternalInput").ap()
    wo_in = nc.dram_tensor("wo", [128, D // 56, 7, 128], BF, kind="ExternalInput").ap()
    out_ap = nc.dram_tensor("out", [D // NC, T], F32, kind="ExternalOutput").ap()
    if DEBUG:
        dbg_attn = nc.dram_tensor("dbg_attn", [HC * DV, T], F32, kind="ExternalOutput").ap()
        dbg_qn = nc.dram_tensor("dbg_qn", [RQ, T], F32, kind="ExternalOutput").ap()
        dbg_kv = nc.dram_tensor("dbg_kv", [RKV, T], F32, kind="ExternalOutput").ap()

    RG = [list(range(NC))]

    with tile.TileContext(nc) as tc:
        es_dram = ExitStack()
        dram = es_dram.enter_context(tc.tile_pool(name="dram", bufs=1, space="DRAM"))
        ar_in = dram.tile([DR, T], F32, tag="ar_in", name="ar_in")
        db_d = dram.tile([HC, T], F32, tag="db_d", name="db_d")
        ar_out = dram.tile([DR, T], F32, tag="ar_out", name="ar_out", addr_space="Shared")
        agq_in = dram.tile([RQ // NC, T], BF, tag="agq_in", name="agq_in")
        agq_out = dram.tile([RQ, T], BF, tag="agq_out", name="agq_out", addr_space="Shared")
        agkv_in = dram.tile([RKV // NC, T], BF, tag="agkv_in", name="agkv_in")
        agkv_out = dram.tile([RKV, T], BF, tag="agkv_out", name="agkv_out", addr_space="Shared")
        ago_in = [dram.tile([4 * DV, T], BF, tag=f"ago_in{i}", name=f"ago_in{i}")
                  for i in range(4)]
        ago_out = [dram.tile([4 * DV * NC, T], BF, tag=f"ago_out{i}",
                             name=f"ago_out{i}", addr_space="Shared")
                   for i in range(4)]

        es_persist = ExitStack()
        persist = es_persist.enter_context(tc.tile_pool(name="persist", bufs=1))
        ones_bf = persist.tile([128, 1], BF, tag="ones", name="ones")
        nc.vector.memset(ones_bf, 1.0)
        eps_t = persist.tile([1, 1], F32, tag="eps", name="eps")
        nc.vector.memset(eps_t, EPS)
        mask_t = persist.tile([128, 128], BF, tag="mask", name="mask")
        make_upper_triangular(nc, mask_t.opt(), val=1.0, diag=True)
        ccss_bf = persist.tile([128, 2 * T], BF, tag="ccss_bf", name="ccss_bf")
        CCb = ccss_bf[:, 0:T]
        SSb = ccss_bf[:, T:2 * T]
        kroped2 = persist.tile([128, T], BF, tag="kroped2", name="kroped2")

        # ============ Stage A: latent projections (sharded) ============
        es_early = ExitStack()
        early = es_early.enter_context(tc.tile_pool(name="early", bufs=1))
        ccss = early.tile([128, 2 * T], F32, tag="ccss", name="ccss")
        nc.sync.dma_start(out=ccss, in_=ccss_in)
        nc.vector.tensor_copy(ccss_bf, ccss)
        CC = ccss[:, 0:T]
        SS = ccss[:, T:2 * T]
        wkpe_t = early.tile([128, 7, DR], BF, tag="wkpe", name="wkpe")
        nc.sync.dma_start(out=wkpe_t, in_=wkpe_in)

        es_xpool = ExitStack()
        xpool = es_xpool.enter_context(tc.tile_pool(name="xpool", bufs=4))
        es_psA = ExitStack()
        psA = es_psA.enter_context(tc.tile_pool(name="psA", bufs=1, space="PSUM"))

        # k_pe partials first: the AllReduce fires early and overlaps the rest
        psk = [psA.tile([64, 512], F32, tag=f"k{i}", name=f"k{i}") for i in range(2)]
        for kt in range(7):
            xk = xpool.tile([128, T], BF, tag="xk", name="xk")
            nc.sync.dma_start(out=xk, in_=xkpe_in[:, kt])
            for ch in range(2):
                nc.tensor.matmul(
                    psk[ch], wkpe_t[:, kt, :], xk[:, 512 * ch:512 * (ch + 1)],
                    start=(kt == 0), stop=(kt == 6))
        kpe_loc = early.tile([64, T], F32, tag="kpe_loc", name="kpe_loc")
        for ch in range(2):
            nc.scalar.copy(kpe_loc[:, 512 * ch:512 * (ch + 1)], psk[ch])
        nc.sync.dma_start(out=ar_in[0:64], in_=kpe_loc)
        nc.gpsimd.collective_compute(
            "AllReduce", mybir.AluOpType.add, replica_groups=RG,
            ins=[ar_in.opt()], outs=[ar_out.opt()])

        # single pass: both a-tiles per x-tile; raw bf16 AllGathers after
        psa = [psA.tile([128, 512], F32, tag=f"a{i}", name=f"a{i}") for i in range(4)]
        for kt in range(KT_X):
            xt = xpool.tile([128, T], BF, tag="x", name="x")
            nc.sync.dma_start(out=xt, in_=x_in[:, kt])
            wa_kt = xpool.tile([128, 2, 128], BF, tag="wa", name="wa")
            nc.sync.dma_start(out=wa_kt, in_=wa_in[:, kt])
            for mt in range(2):
                for ch in range(2):
                    nc.tensor.matmul(
                        psa[mt * 2 + ch], wa_kt[:, mt, :],
                        xt[:, 512 * ch:512 * (ch + 1)],
                        start=(kt == 0), stop=(kt == KT_X - 1))
        t1_bf = early.tile([128, T], BF, tag="t1_bf", name="t1_bf")
        for ch in range(2):
            nc.scalar.copy(t1_bf[:, 512 * ch:512 * (ch + 1)], psa[2 + ch])
        nc.sync.dma_start(out=agkv_in[0:64], in_=t1_bf[64:128])
        nc.gpsimd.collective_compute(
            "AllGather", mybir.AluOpType.bypass, replica_groups=RG,
            ins=[agkv_in.opt()], outs=[agkv_out.opt()])
        t0_bf = early.tile([128, T], BF, tag="t0_bf", name="t0_bf")
        for ch in range(2):
            nc.scalar.copy(t0_bf[:, 512 * ch:512 * (ch + 1)], psa[ch])
        nc.sync.dma_start(out=agq_in[128:192], in_=t1_bf[0:64])
        nc.sync.dma_start(out=agq_in[0:128], in_=t0_bf)
        nc.gpsimd.collective_compute(
            "AllGather", mybir.AluOpType.bypass, replica_groups=RG,
            ins=[agq_in.opt()], outs=[agq_out.opt()])

        # k_pe rope (consumes the AllReduce; off the critical path)
        kpe_sb = early.tile([64, T], F32, tag="kpe_sb", name="kpe_sb")
        nc.sync.dma_start(out=kpe_sb, in_=ar_out[0:64])
        kpe_sw = early.tile([64, T], F32, tag="kpe_sw", name="kpe_sw")
        nc.sync.dma_start(out=kpe_sw[0:32], in_=kpe_sb[32:64])
        nc.sync.dma_start(out=kpe_sw[32:64], in_=kpe_sb[0:32])
        kt1 = early.tile([64, T], F32, tag="kt1", name="kt1")
        kt2 = early.tile([64, T], F32, tag="kt2", name="kt2")
        nc.vector.tensor_mul(kt1, kpe_sb, CC[0:64])
        nc.vector.tensor_mul(kt2, kpe_sw, SS[0:64])
        kroped = early.tile([64, T], BF, tag="kroped", name="kroped")
        nc.vector.tensor_add(kroped, kt1, kt2)
        nc.sync.dma_start(out=kroped2[0:64], in_=kroped)
        nc.sync.dma_start(out=kroped2[64:128], in_=kroped)
        es_psA.close()
        es_xpool.close()
        es_early.close()

        es_heads = ExitStack()
        heads = es_heads.enter_context(tc.tile_pool(name="heads", bufs=1))
        es_qnkv = ExitStack()
        qnkv = es_qnkv.enter_context(tc.tile_pool(name="qnkv", bufs=1))
        es_psQ = ExitStack()
        psQ = es_psQ.enter_context(tc.tile_pool(name="psQ", bufs=1, space="PSUM"))
        es_midkv = ExitStack()
        midkv = es_midkv.enter_context(tc.tile_pool(name="midkv", bufs=1))
        es_sqp = ExitStack()
        sqp = es_sqp.enter_context(tc.tile_pool(name="sqp", bufs=2))

        ckv = [qnkv.tile([128, T], BF, tag=f"ckv{k}", name=f"ckv{k}") for k in range(KT_KV)]
        for k in range(KT_KV):
            nc.sync.dma_start(out=ckv[k], in_=agkv_out[128 * k:128 * (k + 1)])
        pskv = [psQ.tile([1, 512], F32, tag=f"dv{c}", name=f"dv{c}") for c in range(2)]
        for k in range(KT_KV):
            sq = sqp.tile([128, T], BF, tag="sq", name="sq")
            nc.vector.tensor_mul(sq, ckv[k], ckv[k])
            for ch in range(2):
                nc.tensor.matmul(pskv[ch], ones_bf,
                                 sq[:, 512 * ch:512 * (ch + 1)],
                                 start=(k == 0), stop=(k == KT_KV - 1))
        invkv_t = midkv.tile([1, T], F32, tag="invkv_t", name="invkv_t")
        tmp2 = midkv.tile([1, T], F32, tag="tmp2", name="tmp2")
        for ch in range(2):
            cs = slice(512 * ch, 512 * (ch + 1))
            nc.scalar.activation(out=tmp2[:, cs], in_=pskv[ch],
                                 func=mybir.ActivationFunctionType.Sqrt,
                                 bias=eps_t, scale=1.0 / RKV)
        nc.vector.reciprocal(out=invkv_t, in_=tmp2)
        invkv_d = dram.tile([1, T], F32, tag="invkv_d", name="invkv_d")
        nc.sync.dma_start(out=invkv_d[:], in_=invkv_t)
        invkv_b = midkv.tile([128, 1, T], F32, tag="invkv_b", name="invkv_b")
        nc.sync.dma_start(out=invkv_b, in_=invkv_d[:].partition_broadcast(128))
        for k in range(KT_KV):
            nc.vector.tensor_mul(ckv[k], ckv[k], invkv_b[:, 0, :])
        es_sqp.close()
        es_midkv.close()

        qn = [qnkv.tile([128, T], BF, tag=f"qn{k}", name=f"qn{k}") for k in range(KT_Q)]
        for k in range(KT_Q):
            nc.sync.dma_start(out=qn[k], in_=agq_out[128 * k:128 * (k + 1)])

        if DEBUG:
            es_dq = ExitStack()
            dq = es_dq.enter_context(tc.tile_pool(name="dbgq", bufs=2))
            for k in range(KT_Q):
                t = dq.tile([128, T], F32, tag="d", name="d")
                nc.scalar.copy(t, qn[k])
                nc.sync.dma_start(out=dbg_qn[128 * k:128 * (k + 1)], in_=t)
            for k in range(KT_KV):
                t = dq.tile([128, T], F32, tag="d", name="d")
                nc.scalar.copy(t, ckv[k])
                nc.sync.dma_start(out=dbg_kv[128 * k:128 * (k + 1)], in_=t)
            es_dq.close()

        # ============ Stage D: kv_b projections ============
        kn = [heads.tile([128, T], BF, tag=f"kn{m}", name=f"kn{m}") for m in range(HC)]
        v_t = [heads.tile([128, HC * DV], BF, tag=f"v{t_}", name=f"v{t_}") for t_ in range(8)]

        es_s5 = ExitStack()
        s5 = es_s5.enter_context(tc.tile_pool(name="s5", bufs=2))
        es_ps5 = ExitStack()
        ps5 = es_ps5.enter_context(tc.tile_pool(name="ps5", bufs=3, space="PSUM"))
        for mt in range(HC):
            wt = s5.tile([128, KT_KV, 128], BF, tag="wkn", name="wkn")
            nc.sync.dma_start(out=wt, in_=wkn_in[:, mt])
            for ch in range(2):
                ps = ps5.tile([128, 512], F32, tag="ps", name="ps")
                for kt in range(KT_KV):
                    nc.tensor.matmul(ps, wt[:, kt, :],
                                     ckv[kt][:, 512 * ch:512 * (ch + 1)],
                                     start=(kt == 0), stop=(kt == KT_KV - 1))
                nc.scalar.copy(kn[mt][:, 512 * ch:512 * (ch + 1)], ps)
        wv_t = s5.tile([128, KT_KV, HC * DV], BF, tag="wv", name="wv")
        for kt in range(KT_KV):
            nc.sync.dma_start(out=wv_t[:, kt], in_=wv_in[:, kt])
        for tt in range(8):
            for ch in range(4):
                ps = ps5.tile([128, 512], F32, tag="ps", name="ps")
                for kt in range(KT_KV):
                    nc.tensor.matmul(
                        ps, ckv[kt][:, 128 * tt:128 * (tt + 1)],
                        wv_t[:, kt, 512 * ch:512 * (ch + 1)],
                        start=(kt == 0), stop=(kt == KT_KV - 1))
                nc.scalar.copy(v_t[tt][:, 512 * ch:512 * (ch + 1)], ps)
        es_ps5.close()
        es_s5.close()

        # local q sumsq + in-place normalize (replicated, cheap)
        es_midq = ExitStack()
        midq = es_midq.enter_context(tc.tile_pool(name="midq", bufs=1))
        es_sqq = ExitStack()
        sqq = es_sqq.enter_context(tc.tile_pool(name="sqq", bufs=2))
        psqq = [psQ.tile([1, 512], F32, tag=f"dq{c}", name=f"dq{c}") for c in range(2)]
        for k in range(KT_Q):
            sq = sqq.tile([128, T], BF, tag="sq", name="sq")
            nc.vector.tensor_mul(sq, qn[k], qn[k])
            for ch in range(2):
                nc.tensor.matmul(psqq[ch], ones_bf,
                                 sq[:, 512 * ch:512 * (ch + 1)],
                                 start=(k == 0), stop=(k == KT_Q - 1))
        invq_t = midq.tile([1, T], F32, tag="invq_t", name="invq_t")
        tmp1 = midq.tile([1, T], F32, tag="tmp1", name="tmp1")
        for ch in range(2):
            cs = slice(512 * ch, 512 * (ch + 1))
            nc.scalar.activation(out=tmp1[:, cs], in_=psqq[ch],
                                 func=mybir.ActivationFunctionType.Sqrt,
                                 bias=eps_t, scale=1.0 / RQ)
        nc.vector.reciprocal(out=invq_t, in_=tmp1)
        invq_d = dram.tile([1, T], F32, tag="invq_d", name="invq_d")
        nc.sync.dma_start(out=invq_d[:], in_=invq_t)
        invq_b = midq.tile([128, 1, T], F32, tag="invq_b", name="invq_b")
        nc.sync.dma_start(out=invq_b, in_=invq_d[:].partition_broadcast(128))
        for k in range(KT_Q):
            nc.vector.tensor_mul(qn[k], qn[k], invq_b[:, 0, :])
        es_sqq.close()
        es_midq.close()
        es_psQ.close()

        # ============ Stage C: q_b projection + RoPE ============
        qh = [heads.tile([128, T], BF, tag=f"qh{m}", name=f"qh{m}") for m in range(HC)]
        qr = [heads.tile([128, T], BF, tag=f"qr{m}", name=f"qr{m}") for m in range(8)]

        es_s4 = ExitStack()
        s4 = es_s4.enter_context(tc.tile_pool(name="s4", bufs=3))
        es_s4t = ExitStack()
        s4t = es_s4t.enter_context(tc.tile_pool(name="s4t", bufs=2))
        es_ps4 = ExitStack()
        ps4 = es_ps4.enter_context(tc.tile_pool(name="ps4", bufs=4, space="PSUM"))
        for mt in range(MT_QB):
            wt = s4.tile([128, KT_Q, 128], BF, tag="wqb", name="wqb")
            nc.sync.dma_start(out=wt, in_=wqb_in[:, mt])
            pss = []
            for ch in range(2):
                ps = ps4.tile([128, 512], F32, tag="ps", name="ps")
                for kt in range(KT_Q):
                    nc.tensor.matmul(ps, wt[:, kt, :],
                                     qn[kt][:, 512 * ch:512 * (ch + 1)],
                                     start=(kt == 0), stop=(kt == KT_Q - 1))
                pss.append(ps)
            if mt < HC:
                for ch in range(2):
                    nc.scalar.copy(qh[mt][:, 512 * ch:512 * (ch + 1)], pss[ch])
            else:
                rt = mt - HC
                p_bf = s4t.tile([128, T], BF, tag="p_bf", name="p_bf")
                for ch in range(2):
                    nc.scalar.copy(p_bf[:, 512 * ch:512 * (ch + 1)], pss[ch])
                p_sw = s4t.tile([128, T], BF, tag="p_sw", name="p_sw")
                nc.sync.dma_start(out=p_sw[0:32], in_=p_bf[32:64])
                nc.sync.dma_start(out=p_sw[32:64], in_=p_bf[0:32])
                nc.sync.dma_start(out=p_sw[64:96], in_=p_bf[96:128])
                nc.sync.dma_start(out=p_sw[96:128], in_=p_bf[64:96])
                t1 = s4t.tile([128, T], BF, tag="t1", name="t1")
                t2 = s4t.tile([128, T], BF, tag="t2", name="t2")
                nc.vector.tensor_mul(t1, p_bf, CCb)
                nc.vector.tensor_mul(t2, p_sw, SSb)
                nc.vector.tensor_add(qr[rt], t1, t2)
        es_ps4.close()
        es_s4t.close()
        es_s4.close()
        es_qnkv.close()

        # ============ Attention (16 heads) ============
        es_s6 = ExitStack()
        s6 = es_s6.enter_context(tc.tile_pool(name="s6", bufs=4))
        es_s6b = ExitStack()
        s6b = es_s6b.enter_context(tc.tile_pool(name="s6b", bufs=3))
        es_psS = ExitStack()
        psS = es_psS.enter_context(tc.tile_pool(name="psS", bufs=3, space="PSUM"))
        es_psO = ExitStack()
        psO = es_psO.enter_context(tc.tile_pool(name="psO", bufs=2, space="PSUM"))
        es_psD = ExitStack()
        psD = es_psD.enter_context(tc.tile_pool(name="psD", bufs=1, space="PSUM"))
        for hh in range(HC):
            rt, half = hh // 2, hh % 2
            qr_sl = qr[rt][64 * half:64 * (half + 1)]
            kp_sl = kroped2[64 * half:64 * (half + 1)]
            pso = [psO.tile([128, 512], F32, tag=f"o{c}", name=f"o{c}") for c in range(2)]
            acc = s6b.tile([128, T], F32, tag="acc", name="acc")
            acc2 = s6b.tile([128, T], F32, tag="acc2", name="acc2")
            for jt in range(8):
                qlo = 128 * jt
                for ch in range(2):
                    ns, ne = max(qlo, 512 * ch), 512 * (ch + 1)
                    if ns >= ne:
                        continue
                    w = ne - ns
                    pst = psS.tile([128, 512], F32, tag="s", name="s")
                    nc.tensor.matmul(pst[:, 0:w], kn[hh][:, qlo:qlo + 128],
                                     qh[hh][:, ns:ne], start=True, stop=False)
                    nc.tensor.matmul(pst[:, 0:w], kp_sl[:, qlo:qlo + 128],
                                     qr_sl[:, ns:ne], start=False, stop=True)
                    et = s6.tile([128, 512], BF, tag="et", name="et")
                    nc.scalar.activation(out=et[:, 0:w], in_=pst[:, 0:w],
                                         func=mybir.ActivationFunctionType.Exp,
                                         scale=SCALE)
                    if ns == qlo:
                        nc.vector.tensor_mul(et[:, 0:128], et[:, 0:128], mask_t)
                    ost = ns - 512 * ch
                    nc.tensor.matmul(pso[ch][:, ost:512],
                                     v_t[jt][:, 128 * hh:128 * (hh + 1)],
                                     et[:, 0:w], start=(jt == 0), stop=(jt == 7),
                                     skip_group_check=True)
                    dst = acc if jt % 2 == 0 else acc2
                    if jt <= 1:
                        nc.vector.tensor_copy(dst[:, ns:ne], et[:, 0:w])
                    else:
                        nc.vector.tensor_add(dst[:, ns:ne], dst[:, ns:ne],
                                             et[:, 0:w])
            accb = s6b.tile([128, T], BF, tag="accb", name="accb")
            nc.vector.tensor_copy(accb[:, 0:128], acc[:, 0:128])
            nc.vector.tensor_add(accb[:, 128:T], acc[:, 128:T], acc2[:, 128:T])
            rec = s6b.tile([1, T], F32, tag="rec", name="rec")
            obf = s6b.tile([128, T], BF, tag="obf", name="obf")
            for ch in range(2):
                cs = slice(512 * ch, 512 * (ch + 1))
                psd = psD.tile([1, 512], F32, tag="d", name="d")
                nc.tensor.matmul(psd, ones_bf, accb[:, cs], start=True, stop=True)
                nc.vector.reciprocal(rec[:, cs], psd)
            nc.sync.dma_start(out=db_d[hh:hh + 1], in_=rec)
            den_b = s6b.tile([128, 1, T], F32, tag="den_b", name="den_b")
            nc.sync.dma_start(out=den_b,
                              in_=db_d[hh:hh + 1].partition_broadcast(128))
            for ch in range(2):
                cs = slice(512 * ch, 512 * (ch + 1))
                nc.vector.tensor_mul(obf[:, cs], pso[ch], den_b[:, 0, cs])
            nc.sync.dma_start(
                out=ago_in[hh // 4][128 * (hh % 4):128 * (hh % 4 + 1)], in_=obf)
            if DEBUG:
                df = s6b.tile([128, T], F32, tag="dbg", name="dbg")
                nc.scalar.copy(df, obf)
                nc.sync.dma_start(out=dbg_attn[128 * hh:128 * (hh + 1)], in_=df)
            if hh % 4 == 3:
                i = hh // 4
                nc.gpsimd.collective_compute(
                    "AllGather", mybir.AluOpType.bypass, replica_groups=RG,
                    ins=[ago_in[i].opt()], outs=[ago_out[i].opt()])
        es_psD.close()
        es_psO.close()
        es_psS.close()
        es_s6b.close()
        es_s6.close()

        es_heads.close()

        # ============ o_proj (row shard, K = 16384) ============
        es_s7w = ExitStack()
        s7w = es_s7w.enter_context(tc.tile_pool(name="s7w", bufs=6))
        es_s7r = ExitStack()
        s7r = es_s7r.enter_context(tc.tile_pool(name="s7r", bufs=6))
        es_s7o = ExitStack()
        s7o = es_s7o.enter_context(tc.tile_pool(name="s7o", bufs=4))
        es_ps7 = ExitStack()
        ps7 = es_ps7.enter_context(tc.tile_pool(name="ps7", bufs=1, space="PSUM"))
        for ch in range(2):
            pso7 = [ps7.tile([128, 512], F32, tag=f"m{m}", name=f"m{m}")
                    for m in range(7)]
            for i in range(4):
                for c in range(NC):
                    for j in range(4):
                        ktg = 16 * c + 4 * i + j
                        wt = s7w.tile([128, 7, 128], BF, tag="wo", name="wo")
                        nc.sync.dma_start(out=wt, in_=wo_in[:, ktg])
                        rh = s7r.tile([128, 512], BF, tag="rh", name="rh")
                        nc.sync.dma_start(
                            out=rh,
                            in_=ago_out[i][512 * c + 128 * j:512 * c + 128 * (j + 1),
                                           512 * ch:512 * (ch + 1)])
                        st = (i == 0 and c == 0 and j == 0)
                        sp = (i == 3 and c == NC - 1 and j == 3)
                        for mtt in range(7):
                            nc.tensor.matmul(pso7[mtt], wt[:, mtt, :], rh,
                                             start=st, stop=sp)
            for mtt in range(7):
                ot = s7o.tile([128, 512], F32, tag="ot", name="ot")
                nc.scalar.copy(ot, pso7[mtt])
                nc.sync.dma_start(
                    out=out_ap[128 * mtt:128 * (mtt + 1), 512 * ch:512 * (ch + 1)],
                    in_=ot)
        es_ps7.close()
        es_s7o.close()
        es_s7r.close()
        es_s7w.close()

        es_persist.close()
        es_dram.close()

    nc.finalize()
    return nc


def _prep_inputs(hidden_states, cos, sin, wq_a, q_ln_w, wq_b, wkv_a, kv_ln_w,
                 wkv_b, wo):
    """Host-side sharding + layout. Returns in_maps (list of dicts per core)."""
    h2 = np.ascontiguousarray(hidden_states.reshape(S, D).T)      # [D, T]
    xh = np.ascontiguousarray(
        h2.reshape(KT_X, 128, T).transpose(1, 0, 2)).astype(bf16)  # [128,56,T]

    cosT = np.ascontiguousarray(cos.reshape(T, DR).T).astype(np.float32)
    sinT = np.ascontiguousarray(sin.reshape(T, DR).T).astype(np.float32)
    CCh = np.vstack([cosT, cosT])
    SSh = np.vstack([-sinT[:32], sinT[32:], -sinT[:32], sinT[32:]])
    ccss = np.ascontiguousarray(np.hstack([CCh, SSh])).astype(np.float32)

    wq_b_eff = (wq_b * q_ln_w[None, :]).astype(np.float32)
    wkv_b_eff = (wkv_b * kv_ln_w[None, :]).astype(np.float32)
    wq_b_r = wq_b_eff.reshape(H, DQ, RQ)
    wkv_b_r = wkv_b_eff.reshape(H, DN + DV, RKV)
    woT = wo.T                                                    # [16384, D]

    def lhst_tiles(lhsT, kt, mt):
        # [K, M] -> [128, mt, kt, 128]
        K, M = lhsT.shape
        return np.ascontiguousarray(
            lhsT.reshape(kt, 128, mt, 128).transpose(1, 2, 0, 3)).astype(bf16)

    in_maps = []
    for c in range(NC):
        m = {"x": xh, "ccss": ccss}
        m["xkpe"] = np.ascontiguousarray(xh[:, 7 * c:7 * (c + 1), :])
        # stage A slice: 192 q rows + 64 kv rows  -> lhsT [D, 256]
        qs = wq_a[192 * c:192 * (c + 1)]                           # [192, D]
        ks = wkv_a[64 * c:64 * (c + 1)]                            # [64, D]
        lhsT_a = np.vstack([qs, ks]).T                             # [D, 256]
        m["wa"] = lhst_tiles(lhsT_a, KT_X, 2).transpose(0, 2, 1, 3).copy()
        # k_pe K-shard: wkv_a rows 512:576, K cols 896c..
        lhsT_kpe = wkv_a[RKV:RKV + DR, 896 * c:896 * (c + 1)].T    # [896, 64]
        m["wkpe"] = np.ascontiguousarray(
            lhsT_kpe.reshape(7, 128, DR).transpose(1, 0, 2)).astype(bf16)
        hs = slice(HC * c, HC * (c + 1))
        wq_b_c = wq_b_r[hs]                                        # [16,192,RQ]
        lhsT_qb = np.vstack([
            wq_b_c[:, :DN, :].reshape(HC * DN, RQ),
            wq_b_c[:, DN:, :].reshape(HC * DR, RQ)]).T             # [RQ, 3072]
        m["wqb"] = lhst_tiles(lhsT_qb, KT_Q, MT_QB)
        lhsT_kn = wkv_b_r[hs][:, :DN, :].reshape(HC * DN, RKV).T   # [RKV, 2048]
        m["wkn"] = lhst_tiles(lhsT_kn, KT_KV, HC)
        rhs_v = wkv_b_r[hs][:, DN:, :].reshape(HC * DV, RKV).T     # [RKV, 2048]
        m["wv"] = np.ascontiguousarray(
            rhs_v.reshape(KT_KV, 128, HC * DV).transpose(1, 0, 2)).astype(bf16)
        lhsT_wo = woT[:, 896 * c:896 * (c + 1)]                    # [16384, 896]
        m["wo"] = lhst_tiles(lhsT_wo, H * DV // 128, 7).transpose(0, 2, 1, 3).copy()
        in_maps.append(m)
    return in_maps


def _get_nc():
    if "nc" not in _CACHE:
        _CACHE["nc"] = _build()
    return _CACHE["nc"]


def run(in_maps, trace=False, trace_kwargs=None):
    nc = _get_nc()
    return run_bass_kernel_spmd(nc, in_maps, list(range(NC)), trace=trace,
                                **(trace_kwargs or {}))


def kernel(hidden_states, cos, sin, wq_a, q_ln_w, wq_b, wkv_a, kv_ln_w,
           wkv_b, wo):
    in_maps = _prep_inputs(hidden_states, cos, sin, wq_a, q_ln_w, wq_b,
                           wkv_a, kv_ln_w, wkv_b, wo)
    res = run(in_maps)
    out = np.concatenate([res.results[c]["out"] for c in range(NC)], axis=0)
    return np.ascontiguousarray(out.T).reshape(B, S, D).astype(np.float32)



# revision 3
# speedup vs baseline: 1.0887x; 1.0887x over previous
"""DeepSeek-V3 MLA attention (B=1, S=1024, D=7168, H=128) on 8 Trainium2
NeuronCores — v2 (overlap-optimized).

Sharding: tensor-parallel over the 128 heads (16 heads/core); wq_a/wkv_a
M-sharded with bf16 AllGathers of the raw (unnormalized) latents; rmsnorm
sum-of-squares is computed on local shards and combined with a tiny [2,T]
AllReduce that overlaps the AllGathers; k_pe is K-sharded with an early
AllReduce; head outputs AllGathered (bf16) in 4 groups; o_proj row-sharded.

Perf structure vs v1: softmax denominators use a DMA-broadcast + 128-lane
reciprocal (no 1-lane DVE reciprocal, no per-head PE stalls); attention is
software-pipelined with 2-unit lookahead so the in-order PE queue never
waits on the scalar-engine exp; weight streams ride the scalar-engine DMA
queue while x/activations ride sync.
"""

import os
from collections import deque
from contextlib import ExitStack

import numpy as np
import ml_dtypes

import concourse.bass as bass
import concourse.mybir as mybir
import concourse.tile as tile
from concourse import bacc
from concourse.bass_utils import run_bass_kernel_spmd
from concourse.masks import make_upper_triangular

bf16 = ml_dtypes.bfloat16
F32 = mybir.dt.float32
BF = mybir.dt.bfloat16

B, S, D = 1, 1024, 7168
H, DN, DR, DV = 128, 128, 64, 128
DQ = DN + DR                  # 192
RQ, RKV = 1536, 512
EPS = 1e-6
SCALE = float(DQ) ** -0.5
NC = 8
HC = H // NC                  # 16 heads per core
T = S
KT_X = D // 128               # 56
KT_Q = RQ // 128              # 12
KT_KV = RKV // 128            # 4
MT_QB = (HC * DQ) // 128      # 24 (16 nope tiles + 8 rope tiles)

_CACHE = {}


def _build():
    nc = bacc.Bacc("TRN2", target_bir_lowering=False, debug=False, num_devices=NC)

    x_in = nc.dram_tensor("x", [128, KT_X, T], BF, kind="ExternalInput").ap()
    xkpe_in = nc.dram_tensor("xkpe", [128, 7, T], BF, kind="ExternalInput").ap()
    wa_in = nc.dram_tensor("wa", [128, KT_X, 2, 128], BF, kind="ExternalInput").ap()
    wkpe_in = nc.dram_tensor("wkpe", [128, 7, DR], BF, kind="ExternalInput").ap()
    ccss_in = nc.dram_tensor("ccss", [128, 2 * T], F32, kind="ExternalInput").ap()
    wqb_in = nc.dram_tensor("wqb", [128, MT_QB, KT_Q, 128], BF, kind="ExternalInput").ap()
    wkn_in = nc.dram_tensor("wkn", [128, HC, KT_KV, 128], BF, kind="ExternalInput").ap()
    wv_in = nc.dram_tensor("wv", [128, KT_KV, HC * DV], BF, kind="ExternalInput").ap()
    wo_in = nc.dram_tensor("wo", [128, D // 56, 7, 128], BF, kind="ExternalInput").ap()
    out_ap = nc.dram_tensor("out", [D // NC, T], F32, kind="ExternalOutput").ap()

    RG = [list(range(NC))]

    with tile.TileContext(nc) as tc:
        es_dram = ExitStack()
        dram = es_dram.enter_context(tc.tile_pool(name="dram", bufs=1, space="DRAM"))
        ar_in = dram.tile([DR, T], F32, tag="ar_in", name="ar_in")
        ar_out = dram.tile([DR, T], F32, tag="ar_out", name="ar_out", addr_space="Shared")
        ss_in = dram.tile([2, T], F32, tag="ss_in", name="ss_in")
        ss_out = dram.tile([2, T], F32, tag="ss_out", name="ss_out", addr_space="Shared")
        inv_d = dram.tile([2, T], F32, tag="inv_d", name="inv_d")
        db_d = dram.tile([HC, T], F32, tag="db_d", name="db_d")
        agq_in = dram.tile([RQ // NC, T], BF, tag="agq_in", name="agq_in")
        agq_out = dram.tile([RQ, T], BF, tag="agq_out", name="agq_out", addr_space="Shared")
        agkv_in = dram.tile([RKV // NC, T], BF, tag="agkv_in", name="agkv_in")
        agkv_out = dram.tile([RKV, T], BF, tag="agkv_out", name="agkv_out", addr_space="Shared")
        ago_in = [dram.tile([4 * DV, T], BF, tag=f"ago_in{i}", name=f"ago_in{i}")
                  for i in range(4)]
        ago_out = [dram.tile([4 * DV * NC, T], BF, tag=f"ago_out{i}",
                             name=f"ago_out{i}", addr_space="Shared")
                   for i in range(4)]

        es_persist = ExitStack()
        persist = es_persist.enter_context(tc.tile_pool(name="persist", bufs=1))
        ones_bf = persist.tile([128, 1], BF, tag="ones", name="ones")
        nc.vector.memset(ones_bf, 1.0)
        eps_t = persist.tile([1, 1], F32, tag="eps", name="eps")
        nc.vector.memset(eps_t, EPS)
        mask_t = persist.tile([128, 128], BF, tag="mask", name="mask")
        make_upper_triangular(nc, mask_t.opt(), val=1.0, diag=True)
        ccss_bf = persist.tile([128, 2 * T], BF, tag="ccss_bf", name="ccss_bf")
        CCb = ccss_bf[:, 0:T]
        SSb = ccss_bf[:, T:2 * T]
        kroped2 = persist.tile([128, T], BF, tag="kroped2", name="kroped2")

        # ============ Stage A: latent projections (sharded) ============
        es_early = ExitStack()
        early = es_early.enter_context(tc.tile_pool(name="early", bufs=1))
        ccss = early.tile([128, 2 * T], F32, tag="ccss", name="ccss")
        nc.scalar.dma_start(out=ccss, in_=ccss_in)
        nc.vector.tensor_copy(ccss_bf, ccss)
        wkpe_t = early.tile([128, 7, DR], BF, tag="wkpe", name="wkpe")
        nc.scalar.dma_start(out=wkpe_t, in_=wkpe_in)

        es_xpool = ExitStack()
        xpool = es_xpool.enter_context(tc.tile_pool(name="xpool", bufs=8))
        es_wapool = ExitStack()
        wapool = es_wapool.enter_context(tc.tile_pool(name="wapool", bufs=8))
        es_psA = ExitStack()
        psA = es_psA.enter_context(tc.tile_pool(name="psA", bufs=1, space="PSUM"))
        es_psK = ExitStack()
        psK = es_psK.enter_context(tc.tile_pool(name="psK", bufs=1, space="PSUM"))

        # k_pe partials first: the AllReduce fires early and overlaps the rest
        psk = [psK.tile([64, 512], F32, tag=f"k{i}", name=f"k{i}") for i in range(2)]
        for kt in range(7):
            xk = xpool.tile([128, T], BF, tag="x", name="xk")
            nc.sync.dma_start(out=xk, in_=xkpe_in[:, kt])
            for ch in range(2):
                nc.tensor.matmul(
                    psk[ch], wkpe_t[:, kt, :], xk[:, 512 * ch:512 * (ch + 1)],
                    start=(kt == 0), stop=(kt == 6))
        kpe_loc = early.tile([64, T], F32, tag="kpe_loc", name="kpe_loc")
        for ch in range(2):
            nc.scalar.copy(kpe_loc[:, 512 * ch:512 * (ch + 1)], psk[ch])
        nc.sync.dma_start(out=ar_in[0:64], in_=kpe_loc)
        nc.gpsimd.collective_compute(
            "AllReduce", mybir.AluOpType.add, replica_groups=RG,
            ins=[ar_in.opt()], outs=[ar_out.opt()])

        psa = [psA.tile([128, 512], F32, tag=f"a{i}", name=f"a{i}") for i in range(4)]
        for kt in range(KT_X):
            xt = xpool.tile([128, T], BF, tag="x", name="x")
            nc.sync.dma_start(out=xt, in_=x_in[:, kt])
            wa_kt = wapool.tile([128, 2, 128], BF, tag="wa", name="wa")
            nc.scalar.dma_start(out=wa_kt, in_=wa_in[:, kt])
            for mt in range(2):
                for ch in range(2):
                    nc.tensor.matmul(
                        psa[mt * 2 + ch], wa_kt[:, mt, :],
                        xt[:, 512 * ch:512 * (ch + 1)],
                        start=(kt == 0), stop=(kt == KT_X - 1))
        es_psK.close()

        # evacuate; AllGather inputs are the RAW bf16 latents (norm folded later)
        t1_bf = early.tile([128, T], BF, tag="t1_bf", name="t1_bf")
        for ch in range(2):
            nc.scalar.copy(t1_bf[:, 512 * ch:512 * (ch + 1)], psa[2 + ch])
        nc.sync.dma_start(out=agkv_in[0:64], in_=t1_bf[64:128])
        t0_bf = early.tile([128, T], BF, tag="t0_bf", name="t0_bf")
        for ch in range(2):
            nc.scalar.copy(t0_bf[:, 512 * ch:512 * (ch + 1)], psa[ch])
        nc.sync.dma_start(out=agq_in[128:192], in_=t1_bf[0:64])
        nc.sync.dma_start(out=agq_in[0:128], in_=t0_bf)

        # local sum-of-squares partials (q rows: t0 all + t1[0:64]; kv: t1[64:128])
        es_sqp = ExitStack()
        sqp = es_sqp.enter_context(tc.tile_pool(name="sqp", bufs=2))
        es_psSS = ExitStack()
        psSS = es_psSS.enter_context(tc.tile_pool(name="psSS", bufs=1, space="PSUM"))
        ssq = [psSS.tile([1, 512], F32, tag=f"sq{c}", name=f"sq{c}") for c in range(2)]
        ssk = [psSS.tile([1, 512], F32, tag=f"sk{c}", name=f"sk{c}") for c in range(2)]
        sq1 = sqp.tile([128, T], BF, tag="sq", name="sq1")
        nc.vector.tensor_mul(sq1, t1_bf, t1_bf)
        sq0 = sqp.tile([128, T], BF, tag="sq", name="sq0")
        nc.vector.tensor_mul(sq0, t0_bf, t0_bf)
        for ch in range(2):
            cs = slice(512 * ch, 512 * (ch + 1))
            nc.tensor.matmul(ssq[ch], ones_bf, sq0[:, cs], start=True, stop=False)
            nc.tensor.matmul(ssq[ch], ones_bf[0:64], sq1[0:64, cs],
                             start=False, stop=True)
            nc.tensor.matmul(ssk[ch], ones_bf[64:128], sq1[64:128, cs],
                             start=True, stop=True)
        ssq_sb = early.tile([1, T], F32, tag="ssq_sb", name="ssq_sb")
        ssk_sb = early.tile([1, T], F32, tag="ssk_sb", name="ssk_sb")
        for ch in range(2):
            cs = slice(512 * ch, 512 * (ch + 1))
            nc.scalar.copy(ssq_sb[:, cs], ssq[ch])
            nc.scalar.copy(ssk_sb[:, cs], ssk[ch])
        nc.sync.dma_start(out=ss_in[0:1], in_=ssq_sb)
        nc.sync.dma_start(out=ss_in[1:2], in_=ssk_sb)
        nc.gpsimd.collective_compute(
            "AllReduce", mybir.AluOpType.add, replica_groups=RG,
            ins=[ss_in.opt()], outs=[ss_out.opt()])
        nc.gpsimd.collective_compute(
            "AllGather", mybir.AluOpType.bypass, replica_groups=RG,
            ins=[agkv_in.opt()], outs=[agkv_out.opt()])
        nc.gpsimd.collective_compute(
            "AllGather", mybir.AluOpType.bypass, replica_groups=RG,
            ins=[agq_in.opt()], outs=[agq_out.opt()])
        es_psSS.close()
        es_sqp.close()
        es_psA.close()
        es_wapool.close()
        es_xpool.close()
        es_early.close()

        # ---- inv scales via tiny AR result ----
        es_mid = ExitStack()
        mid = es_mid.enter_context(tc.tile_pool(name="mid", bufs=1))
        es_mid2 = ExitStack()
        mid2 = es_mid2.enter_context(tc.tile_pool(name="mid2", bufs=1))
        ssar_q = mid2.tile([1, T], F32, tag="ssar_q", name="ssar_q")
        nc.sync.dma_start(out=ssar_q, in_=ss_out[0:1])
        ssar_k = mid2.tile([1, T], F32, tag="ssar_k", name="ssar_k")
        nc.sync.dma_start(out=ssar_k, in_=ss_out[1:2])
        rtq_sb = mid2.tile([1, T], F32, tag="rtq_sb", name="rtq_sb")
        nc.scalar.activation(out=rtq_sb, in_=ssar_q,
                             func=mybir.ActivationFunctionType.Sqrt,
                             bias=eps_t, scale=1.0 / RQ)
        rtk_sb = mid2.tile([1, T], F32, tag="rtk_sb", name="rtk_sb")
        nc.scalar.activation(out=rtk_sb, in_=ssar_k,
                             func=mybir.ActivationFunctionType.Sqrt,
                             bias=eps_t, scale=1.0 / RKV)
        invq_sb = mid2.tile([1, T], F32, tag="invq_sb", name="invq_sb")
        nc.vector.reciprocal(out=invq_sb, in_=rtq_sb)
        invkv_sb = mid2.tile([1, T], F32, tag="invkv_sb", name="invkv_sb")
        nc.vector.reciprocal(out=invkv_sb, in_=rtk_sb)
        nc.sync.dma_start(out=inv_d[0:1], in_=invq_sb)
        nc.sync.dma_start(out=inv_d[1:2], in_=invkv_sb)
        invq_b = mid.tile([128, 1, T], F32, tag="invq_b", name="invq_b")
        nc.sync.dma_start(out=invq_b, in_=inv_d[0:1].partition_broadcast(128))
        invkv_b = mid.tile([128, 1, T], F32, tag="invkv_b", name="invkv_b")
        nc.sync.dma_start(out=invkv_b, in_=inv_d[1:2].partition_broadcast(128))

        # k_pe rope (consumes the kpe AllReduce; off the critical path)
        kpe_sb = mid2.tile([64, T], F32, tag="kpe_sb", name="kpe_sb")
        nc.sync.dma_start(out=kpe_sb, in_=ar_out[0:64])
        kpe_sw = mid2.tile([64, T], F32, tag="kpe_sw", name="kpe_sw")
        nc.sync.dma_start(out=kpe_sw[0:32], in_=kpe_sb[32:64])
        nc.sync.dma_start(out=kpe_sw[32:64], in_=kpe_sb[0:32])
        kt1 = mid2.tile([64, T], F32, tag="kt1", name="kt1")
        kt2 = mid2.tile([64, T], F32, tag="kt2", name="kt2")
        nc.vector.tensor_mul(kt1, kpe_sb, CCb[0:64])
        nc.vector.tensor_mul(kt2, kpe_sw, SSb[0:64])
        kroped = mid2.tile([64, T], BF, tag="kroped", name="kroped")
        nc.vector.tensor_add(kroped, kt1, kt2)
        nc.sync.dma_start(out=kroped2[0:64], in_=kroped)
        nc.sync.dma_start(out=kroped2[64:128], in_=kroped)
        es_mid2.close()

        es_heads = ExitStack()
        heads = es_heads.enter_context(tc.tile_pool(name="heads", bufs=1))
        es_qnkv = ExitStack()
        qnkv = es_qnkv.enter_context(tc.tile_pool(name="qnkv", bufs=1))

        # ============ Stage B: kv_b projections ============
        ckv = [qnkv.tile([128, T], BF, tag=f"ckv{k}", name=f"ckv{k}") for k in range(KT_KV)]
        for k in range(KT_KV):
            nc.sync.dma_start(out=ckv[k], in_=agkv_out[128 * k:128 * (k + 1)])
            nc.vector.tensor_mul(ckv[k], ckv[k], invkv_b[:, 0, :])

        kn = [heads.tile([128, T], BF, tag=f"kn{m}", name=f"kn{m}") for m in range(HC)]
        v_t = [heads.tile([128, HC * DV], BF, tag=f"v{t_}", name=f"v{t_}") for t_ in range(8)]

        es_s5 = ExitStack()
        s5 = es_s5.enter_context(tc.tile_pool(name="s5", bufs=3))
        es_ps5 = ExitStack()
        ps5 = es_ps5.enter_context(tc.tile_pool(name="ps5", bufs=3, space="PSUM"))
        for mt in range(HC):
            wt = s5.tile([128, KT_KV, 128], BF, tag="wkn", name="wkn")
            nc.scalar.dma_start(out=wt, in_=wkn_in[:, mt])
            pss = [ps5.tile([128, 512], F32, tag="ps", name="ps") for _ in range(2)]
            for kt in range(KT_KV):
                for ch in range(2):
                    nc.tensor.matmul(pss[ch], wt[:, kt, :],
                                     ckv[kt][:, 512 * ch:512 * (ch + 1)],
                                     start=(kt == 0), stop=(kt == KT_KV - 1))
            for ch in range(2):
                nc.scalar.copy(kn[mt][:, 512 * ch:512 * (ch + 1)], pss[ch])
        es_wvp = ExitStack()
        wvp = es_wvp.enter_context(tc.tile_pool(name="wvp", bufs=1))
        wv_t = wvp.tile([128, KT_KV, HC * DV], BF, tag="wv", name="wv")
        for kt in range(KT_KV):
            nc.scalar.dma_start(out=wv_t[:, kt], in_=wv_in[:, kt])
        for tt in range(8):
            for half in range(2):
                pss = [ps5.tile([128, 512], F32, tag="ps", name="ps") for _ in range(2)]
                for kt in range(KT_KV):
                    for c2 in range(2):
                        ch = 2 * half + c2
                        nc.tensor.matmul(
                            pss[c2], ckv[kt][:, 128 * tt:128 * (tt + 1)],
                            wv_t[:, kt, 512 * ch:512 * (ch + 1)],
                            start=(kt == 0), stop=(kt == KT_KV - 1))
                for c2 in range(2):
                    ch = 2 * half + c2
                    nc.scalar.copy(v_t[tt][:, 512 * ch:512 * (ch + 1)], pss[c2])
        es_wvp.close()
        es_ps5.close()
        es_s5.close()

        # ============ Stage C: q_b projection + RoPE ============
        qn = [qnkv.tile([128, T], BF, tag=f"qn{k}", name=f"qn{k}") for k in range(KT_Q)]
        for k in range(KT_Q):
            nc.sync.dma_start(out=qn[k], in_=agq_out[128 * k:128 * (k + 1)])
            nc.vector.tensor_mul(qn[k], qn[k], invq_b[:, 0, :])

        qh = [heads.tile([128, T], BF, tag=f"qh{m}", name=f"qh{m}") for m in range(HC)]
        qr = [heads.tile([128, T], BF, tag=f"qr{m}", name=f"qr{m}") for m in range(8)]

        es_s4 = ExitStack()
        s4 = es_s4.enter_context(tc.tile_pool(name="s4", bufs=4))
        es_s4t = ExitStack()
        s4t = es_s4t.enter_context(tc.tile_pool(name="s4t", bufs=2))
        es_ps4 = ExitStack()
        ps4 = es_ps4.enter_context(tc.tile_pool(name="ps4", bufs=4, space="PSUM"))
        # rope tiles (mt 16..23) first so attention can start right after qh
        for mt in list(range(HC, MT_QB)) + list(range(HC)):
            wt = s4.tile([128, KT_Q, 128], BF, tag="wqb", name="wqb")
            nc.scalar.dma_start(out=wt, in_=wqb_in[:, mt])
            pss = [ps4.tile([128, 512], F32, tag="ps", name="ps") for _ in range(2)]
            for kt in range(KT_Q):
                for ch in range(2):
                    nc.tensor.matmul(pss[ch], wt[:, kt, :],
                                     qn[kt][:, 512 * ch:512 * (ch + 1)],
                                     start=(kt == 0), stop=(kt == KT_Q - 1))
            if mt < HC:
                for ch in range(2):
                    nc.scalar.copy(qh[mt][:, 512 * ch:512 * (ch + 1)], pss[ch])
            else:
                rt = mt - HC
                p_bf = s4t.tile([128, T], BF, tag="p_bf", name="p_bf")
                for ch in range(2):
                    nc.scalar.copy(p_bf[:, 512 * ch:512 * (ch + 1)], pss[ch])
                p_sw = s4t.tile([128, T], BF, tag="p_sw", name="p_sw")
                nc.sync.dma_start(out=p_sw[0:32], in_=p_bf[32:64])
                nc.sync.dma_start(out=p_sw[32:64], in_=p_bf[0:32])
                nc.sync.dma_start(out=p_sw[64:96], in_=p_bf[96:128])
                nc.sync.dma_start(out=p_sw[96:128], in_=p_bf[64:96])
                t1 = s4t.tile([128, T], BF, tag="t1", name="t1")
                t2 = s4t.tile([128, T], BF, tag="t2", name="t2")
                nc.vector.tensor_mul(t1, p_bf, CCb)
                nc.vector.tensor_mul(t2, p_sw, SSb)
                nc.vector.tensor_add(qr[rt], t1, t2)
        es_ps4.close()
        es_s4t.close()
        es_s4.close()
        es_qnkv.close()

        # ============ Attention (16 heads, software-pipelined) ============
        es_s6 = ExitStack()
        s6 = es_s6.enter_context(tc.tile_pool(name="s6", bufs=5))
        es_s6a = ExitStack()
        s6a = es_s6a.enter_context(tc.tile_pool(name="s6a", bufs=4))
        es_s6b = ExitStack()
        s6b = es_s6b.enter_context(tc.tile_pool(name="s6b", bufs=2))
        es_s6d = ExitStack()
        s6d = es_s6d.enter_context(tc.tile_pool(name="s6d", bufs=3))
        es_psS = ExitStack()
        psS = es_psS.enter_context(tc.tile_pool(name="psS", bufs=3, space="PSUM"))
        es_psO = ExitStack()
        psO = es_psO.enter_context(tc.tile_pool(name="psO", bufs=3, space="PSUM"))
        es_psD = ExitStack()
        psD = es_psD.enter_context(tc.tile_pool(name="psD", bufs=2, space="PSUM"))

        # unit list: (hh, ch, jt, ns, ne) with group boundaries at (hh, ch)
        units = []
        for hh in range(HC):
            for ch in range(2):
                for jt in range(8):
                    ns, ne = max(128 * jt, 512 * ch), 512 * (ch + 1)
                    if ns < ne:
                        units.append((hh, ch, jt, ns, ne))

        gstate = {}   # (hh,ch) -> dict(pso, acc)
        inflight = deque()   # (unit, et_tile)
        gdone = deque()      # (key, enqueue_step): groups awaiting den drain
        step = [0]

        def issue(u):
            hh, ch, jt, ns, ne = u
            w = ne - ns
            rt, half = hh // 2, hh % 2
            qr_sl = qr[rt][64 * half:64 * (half + 1)]
            kp_sl = kroped2[64 * half:64 * (half + 1)]
            qlo = 128 * jt
            pst = psS.tile([128, 512], F32, tag="s", name="s")
            nc.tensor.matmul(pst[:, 0:w], kn[hh][:, qlo:qlo + 128],
                             qh[hh][:, ns:ne], start=True, stop=False)
            nc.tensor.matmul(pst[:, 0:w], kp_sl[:, qlo:qlo + 128],
                             qr_sl[:, ns:ne], start=False, stop=True)
            et = s6.tile([128, 512], BF, tag="et", name="et")
            nc.scalar.activation(out=et[:, 0:w], in_=pst[:, 0:w],
                                 func=mybir.ActivationFunctionType.Exp,
                                 scale=SCALE)
            if ns == qlo:
                nc.vector.tensor_mul(et[:, 0:128], et[:, 0:128], mask_t)
            inflight.append((u, et))

        def drain_group(force=False):
            if not gdone:
                return
            if not force and gdone[0][1] >= step[0]:
                return
            (hh, ch), _ = gdone.popleft()
            g = gstate.pop((hh, ch))
            cs = slice(512 * ch, 512 * (ch + 1))
            accb = s6.tile([128, 512], BF, tag="accb", name="accb")
            nc.vector.tensor_copy(accb, g["acc"])
            psd = psD.tile([1, 512], F32, tag="d", name="d")
            nc.tensor.matmul(psd, ones_bf, accb, start=True, stop=True)
            den_sb = s6d.tile([1, 512], F32, tag="den", name="den")
            nc.scalar.copy(den_sb, psd)
            nc.sync.dma_start(out=db_d[hh:hh + 1, cs], in_=den_sb)
            den_bc = s6b.tile([128, 1, 512], F32, tag="dbc", name="dbc")
            nc.sync.dma_start(out=den_bc,
                              in_=db_d[hh:hh + 1, cs].partition_broadcast(128))
            rec_bc = s6b.tile([128, 512], F32, tag="rbc", name="rbc")
            nc.vector.reciprocal_approx_fast(out=rec_bc, in_=den_bc[:, 0, :])
            obf = s6d.tile([128, 512], BF, tag="obf", name="obf")
            nc.vector.tensor_mul(obf, g["pso"], rec_bc)
            nc.sync.dma_start(
                out=ago_in[hh // 4][128 * (hh % 4):128 * (hh % 4 + 1), cs],
                in_=obf)
            if ch == 1 and hh % 4 == 3:
                i = hh // 4
                nc.gpsimd.collective_compute(
                    "AllGather", mybir.AluOpType.bypass, replica_groups=RG,
                    ins=[ago_in[i].opt()], outs=[ago_out[i].opt()])

        def finalize():
            u, et = inflight.popleft()
            hh, ch, jt, ns, ne = u
            w = ne - ns
            key = (hh, ch)
            first = key not in gstate
            if first:
                gstate[key] = dict(
                    pso=psO.tile([128, 512], F32, tag="o", name="o"),
                    acc=s6a.tile([128, 512], F32, tag="acc", name="acc"))
            g = gstate[key]
            os_ = ns - 512 * ch
            last = (jt == 3) if ch == 0 else (jt == 7)
            nc.tensor.matmul(g["pso"][:, os_:512],
                             v_t[jt][:, 128 * hh:128 * (hh + 1)],
                             et[:, 0:w], start=first, stop=last,
                             skip_group_check=True)
            if first:
                nc.vector.tensor_copy(g["acc"], et[:, 0:w])
            else:
                nc.vector.tensor_add(g["acc"][:, os_:512], g["acc"][:, os_:512],
                                     et[:, 0:w])
            if last:
                gdone.append((key, step[0]))

        LOOKAHEAD = 2
        for idx, u in enumerate(units):
            issue(u)
            if idx >= LOOKAHEAD:
                finalize()
            drain_group()
            step[0] += 1
        while inflight:
            finalize()
            drain_group()
            step[0] += 1
        while gdone:
            drain_group(force=True)

        es_psD.close()
        es_psO.close()
        es_psS.close()
        es_s6d.close()
        es_s6b.close()
        es_s6a.close()
        es_s6.close()
        es_heads.close()
        es_mid.close()

        # ============ o_proj (row shard, K = 16384) ============
        es_s7w = ExitStack()
        s7w = es_s7w.enter_context(tc.tile_pool(name="s7w", bufs=6))
        es_s7r = ExitStack()
        s7r = es_s7r.enter_context(tc.tile_pool(name="s7r", bufs=6))
        es_s7o = ExitStack()
        s7o = es_s7o.enter_context(tc.tile_pool(name="s7o", bufs=4))
        es_ps7 = ExitStack()
        ps7 = es_ps7.enter_context(tc.tile_pool(name="ps7", bufs=1, space="PSUM"))
        for ch in range(2):
            pso7 = [ps7.tile([128, 512], F32, tag=f"m{m}", name=f"m{m}")
                    for m in range(7)]
            for i in range(4):
                for c in range(NC):
                    for j in range(4):
                        ktg = 16 * c + 4 * i + j
                        wt = s7w.tile([128, 7, 128], BF, tag="wo", name="wo")
                        nc.scalar.dma_start(out=wt, in_=wo_in[:, ktg])
                        rh = s7r.tile([128, 512], BF, tag="rh", name="rh")
                        nc.sync.dma_start(
                            out=rh,
                            in_=ago_out[i][512 * c + 128 * j:512 * c + 128 * (j + 1),
                                           512 * ch:512 * (ch + 1)])
                        st = (i == 0 and c == 0 and j == 0)
                        sp = (i == 3 and c == NC - 1 and j == 3)
                        for mtt in range(7):
                            nc.tensor.matmul(pso7[mtt], wt[:, mtt, :], rh,
                                             start=st, stop=sp)
            for mtt in range(7):
                ot = s7o.tile([128, 512], F32, tag="ot", name="ot")
                nc.scalar.copy(ot, pso7[mtt])
                nc.sync.dma_start(
                    out=out_ap[128 * mtt:128 * (mtt + 1), 512 * ch:512 * (ch + 1)],
                    in_=ot)
        es_ps7.close()
        es_s7o.close()
        es_s7r.close()
        es_s7w.close()

        es_persist.close()
        es_dram.close()

    nc.finalize()
    return nc


# revision 7
# speedup vs baseline: 1.0989x; 1.0094x over previous
"""DeepSeek-V3 MLA attention (B=1, S=1024, D=7168, H=128) on 8 Trainium2
NeuronCores — v2 (overlap-optimized).

Sharding: tensor-parallel over the 128 heads (16 heads/core); wq_a/wkv_a
M-sharded with bf16 AllGathers of the raw (unnormalized) latents; rmsnorm
sum-of-squares is computed on local shards and combined with a tiny [2,T]
AllReduce that overlaps the AllGathers; k_pe is K-sharded with an early
AllReduce; head outputs AllGathered (bf16) in 4 groups; o_proj row-sharded.

Perf structure vs v1: softmax denominators use a DMA-broadcast + 128-lane
reciprocal (no 1-lane DVE reciprocal, no per-head PE stalls); attention is
software-pipelined with 2-unit lookahead so the in-order PE queue never
waits on the scalar-engine exp; weight streams ride the scalar-engine DMA
queue while x/activations ride sync.
"""

import os
from collections import deque
from contextlib import ExitStack

import numpy as np
import ml_dtypes

import concourse.bass as bass
import concourse.mybir as mybir
import concourse.tile as tile
from concourse import bacc
from concourse.bass_utils import run_bass_kernel_spmd
from concourse.masks import make_upper_triangular

bf16 = ml_dtypes.bfloat16
F32 = mybir.dt.float32
BF = mybir.dt.bfloat16

B, S, D = 1, 1024, 7168
H, DN, DR, DV = 128, 128, 64, 128
DQ = DN + DR                  # 192
RQ, RKV = 1536, 512
EPS = 1e-6
SCALE = float(DQ) ** -0.5
NC = 8
HC = H // NC                  # 16 heads per core
T = S
KT_X = D // 128               # 56
KT_Q = RQ // 128              # 12
KT_KV = RKV // 128            # 4
MT_QB = (HC * DQ) // 128      # 24 (16 nope tiles + 8 rope tiles)

_CACHE = {}


def _build():
    nc = bacc.Bacc("TRN2", target_bir_lowering=False, debug=False, num_devices=NC)

    x_in = nc.dram_tensor("x", [128, KT_X, T], BF, kind="ExternalInput").ap()
    xkpe_in = nc.dram_tensor("xkpe", [128, 7, T], BF, kind="ExternalInput").ap()
    wa_in = nc.dram_tensor("wa", [128, KT_X, 2, 128], BF, kind="ExternalInput").ap()
    wkpe_in = nc.dram_tensor("wkpe", [128, 7, DR], BF, kind="ExternalInput").ap()
    ccss_in = nc.dram_tensor("ccss", [128, 2 * T], F32, kind="ExternalInput").ap()
    wqb_in = nc.dram_tensor("wqb", [128, MT_QB, KT_Q, 128], BF, kind="ExternalInput").ap()
    wkn_in = nc.dram_tensor("wkn", [128, HC, KT_KV, 128], BF, kind="ExternalInput").ap()
    wv_in = nc.dram_tensor("wv", [128, KT_KV, HC * DV], BF, kind="ExternalInput").ap()
    wo_in = nc.dram_tensor("wo", [128, D // 56, 7, 128], BF, kind="ExternalInput").ap()
    out_ap = nc.dram_tensor("out", [D // NC, T], F32, kind="ExternalOutput").ap()

    RG = [list(range(NC))]

    with tile.TileContext(nc) as tc:
        es_dram = ExitStack()
        dram = es_dram.enter_context(tc.tile_pool(name="dram", bufs=1, space="DRAM"))
        ar_in = dram.tile([DR, T], F32, tag="ar_in", name="ar_in")
        ar_out = dram.tile([DR, T], F32, tag="ar_out", name="ar_out", addr_space="Shared")
        agq_in = dram.tile([RQ // NC, T], BF, tag="agq_in", name="agq_in")
        agq_out = dram.tile([RQ, T], BF, tag="agq_out", name="agq_out", addr_space="Shared")
        agkv_in = dram.tile([66, T], BF, tag="agkv_in", name="agkv_in")
        agkv_out = dram.tile([66 * NC, T], BF, tag="agkv_out", name="agkv_out", addr_space="Shared")
        ago_in = [dram.tile([4 * DV, T], BF, tag=f"ago_in{i}", name=f"ago_in{i}")
                  for i in range(4)]
        ago_out = [dram.tile([4 * DV * NC, T], BF, tag=f"ago_out{i}",
                             name=f"ago_out{i}", addr_space="Shared")
                   for i in range(4)]

        es_persist = ExitStack()
        persist = es_persist.enter_context(tc.tile_pool(name="persist", bufs=1))
        ones_bf = persist.tile([128, 1], BF, tag="ones", name="ones")
        nc.vector.memset(ones_bf, 1.0)
        eps_t = persist.tile([1, 1], F32, tag="eps", name="eps")
        nc.vector.memset(eps_t, EPS)
        ones128 = persist.tile([128, 128], BF, tag="ones128", name="ones128")
        nc.vector.memset(ones128, 1.0)
        onesr_f = persist.tile([1, 128], BF, tag="onesr_f", name="onesr_f")
        nc.vector.memset(onesr_f, 1.0)
        mask_t = persist.tile([128, 128], BF, tag="mask", name="mask")
        make_upper_triangular(nc, mask_t.opt(), val=1.0, diag=True)
        ccss_bf = persist.tile([128, 2 * T], BF, tag="ccss_bf", name="ccss_bf")
        CCb = ccss_bf[:, 0:T]
        SSb = ccss_bf[:, T:2 * T]
        kroped2 = persist.tile([128, T], BF, tag="kroped2", name="kroped2")

        # ============ Stage A: latent projections (sharded) ============
        es_early = ExitStack()
        early = es_early.enter_context(tc.tile_pool(name="early", bufs=1))
        ccss = early.tile([128, 2 * T], F32, tag="ccss", name="ccss")
        nc.scalar.dma_start(out=ccss, in_=ccss_in)
        nc.vector.tensor_copy(ccss_bf, ccss)
        wkpe_t = early.tile([128, 7, DR], BF, tag="wkpe", name="wkpe")
        nc.scalar.dma_start(out=wkpe_t, in_=wkpe_in)

        es_xpool = ExitStack()
        xpool = es_xpool.enter_context(tc.tile_pool(name="xpool", bufs=12))
        es_wapool = ExitStack()
        wapool = es_wapool.enter_context(tc.tile_pool(name="wapool", bufs=12))
        es_psA = ExitStack()
        psA = es_psA.enter_context(tc.tile_pool(name="psA", bufs=1, space="PSUM"))
        es_psK = ExitStack()
        psK = es_psK.enter_context(tc.tile_pool(name="psK", bufs=1, space="PSUM"))

        # k_pe partials first: the AllReduce fires early and overlaps the rest
        psk = [psK.tile([64, 512], F32, tag=f"k{i}", name=f"k{i}") for i in range(2)]
        for kt in range(7):
            xk = xpool.tile([128, T], BF, tag="x", name="xk")
            nc.sync.dma_start(out=xk, in_=xkpe_in[:, kt])
            for ch in range(2):
                nc.tensor.matmul(
                    psk[ch], wkpe_t[:, kt, :], xk[:, 512 * ch:512 * (ch + 1)],
                    start=(kt == 0), stop=(kt == 6))
        kpe_loc = early.tile([64, T], F32, tag="kpe_loc", name="kpe_loc")
        for ch in range(2):
            nc.scalar.copy(kpe_loc[:, 512 * ch:512 * (ch + 1)], psk[ch])
        nc.sync.dma_start(out=ar_in[0:64], in_=kpe_loc)
        nc.gpsimd.collective_compute(
            "AllReduce", mybir.AluOpType.add, replica_groups=RG,
            ins=[ar_in.opt()], outs=[ar_out.opt()])

        psa = [psA.tile([128, 512], F32, tag=f"a{i}", name=f"a{i}") for i in range(4)]
        for kt in range(KT_X):
            xt = xpool.tile([128, T], BF, tag="x", name="x")
            nc.sync.dma_start(out=xt, in_=x_in[:, kt])
            wa_kt = wapool.tile([128, 2, 128], BF, tag="wa", name="wa")
            nc.scalar.dma_start(out=wa_kt, in_=wa_in[:, kt])
            for mt in range(2):
                for ch in range(2):
                    nc.tensor.matmul(
                        psa[mt * 2 + ch], wa_kt[:, mt, :],
                        xt[:, 512 * ch:512 * (ch + 1)],
                        start=(kt == 0), stop=(kt == KT_X - 1))
        es_psK.close()

        # evacuate; AllGather inputs are the RAW bf16 latents (norm folded later)
        t1_bf = early.tile([128, T], BF, tag="t1_bf", name="t1_bf")
        for ch in range(2):
            nc.scalar.copy(t1_bf[:, 512 * ch:512 * (ch + 1)], psa[2 + ch])
        nc.sync.dma_start(out=agkv_in[0:64], in_=t1_bf[64:128])

        # local sum-of-squares partials, squared straight out of PSUM on DVE
        # (q rows: t0 all + t1[0:64]; kv: t1[64:128]); ride the agkv AllGather
        es_sqp = ExitStack()
        sqp = es_sqp.enter_context(tc.tile_pool(name="sqp", bufs=2))
        es_psSS = ExitStack()
        psSS = es_psSS.enter_context(tc.tile_pool(name="psSS", bufs=1, space="PSUM"))
        ssq = [psSS.tile([1, 512], F32, tag=f"sq{c}", name=f"sq{c}") for c in range(2)]
        ssk = [psSS.tile([1, 512], F32, tag=f"sk{c}", name=f"sk{c}") for c in range(2)]
        sq1 = sqp.tile([128, T], BF, tag="sq", name="sq1")
        nc.vector.tensor_mul(sq1, t1_bf, t1_bf)
        t0_bf = early.tile([128, T], BF, tag="t0_bf", name="t0_bf")
        for ch in range(2):
            nc.scalar.copy(t0_bf[:, 512 * ch:512 * (ch + 1)], psa[ch])
        nc.sync.dma_start(out=agq_in[128:192], in_=t1_bf[0:64])
        nc.sync.dma_start(out=agq_in[0:128], in_=t0_bf)
        sq0 = sqp.tile([128, T], BF, tag="sq", name="sq0")
        nc.vector.tensor_mul(sq0, t0_bf, t0_bf)
        for ch in range(2):
            cs = slice(512 * ch, 512 * (ch + 1))
            nc.tensor.matmul(ssq[ch], ones_bf[0:64], sq1[0:64, cs],
                             start=True, stop=False)
            nc.tensor.matmul(ssq[ch], ones_bf, sq0[:, cs], start=False, stop=True)
            nc.tensor.matmul(ssk[ch], ones_bf[64:128], sq1[64:128, cs],
                             start=True, stop=True)
        ssq_sb = early.tile([1, T], BF, tag="ssq_sb", name="ssq_sb")
        ssk_sb = early.tile([1, T], BF, tag="ssk_sb", name="ssk_sb")
        for ch in range(2):
            cs = slice(512 * ch, 512 * (ch + 1))
            nc.scalar.copy(ssq_sb[:, cs], ssq[ch])
            nc.scalar.copy(ssk_sb[:, cs], ssk[ch])
        nc.sync.dma_start(out=agkv_in[64:65], in_=ssq_sb)
        nc.sync.dma_start(out=agkv_in[65:66], in_=ssk_sb)
        nc.gpsimd.collective_compute(
            "AllGather", mybir.AluOpType.bypass, replica_groups=RG,
            ins=[agkv_in.opt()], outs=[agkv_out.opt()])
        nc.gpsimd.collective_compute(
            "AllGather", mybir.AluOpType.bypass, replica_groups=RG,
            ins=[agq_in.opt()], outs=[agq_out.opt()])
        es_psSS.close()
        es_sqp.close()
        es_psA.close()
        es_wapool.close()
        es_xpool.close()
        es_early.close()

        # ---- inv scales: sum the gathered bf16 ss partials, rsqrt, then
        # broadcast across partitions with K=1 fp32 matmuls into PSUM ----
        es_heads = ExitStack()
        heads = es_heads.enter_context(tc.tile_pool(name="heads", bufs=1))
        es_qnkv = ExitStack()
        qnkv = es_qnkv.enter_context(tc.tile_pool(name="qnkv", bufs=1))
        es_psInv = ExitStack()
        psInv = es_psInv.enter_context(tc.tile_pool(name="psInv", bufs=1, space="PSUM"))
        es_mid2 = ExitStack()
        mid2 = es_mid2.enter_context(tc.tile_pool(name="mid2", bufs=1))

        ssq_t = mid2.tile([NC, T], BF, tag="ssq_t", name="ssq_t")
        ssk_t = mid2.tile([NC, T], BF, tag="ssk_t", name="ssk_t")
        for r in range(NC):
            nc.sync.dma_start(out=ssq_t[r:r + 1], in_=agkv_out[66 * r + 64:66 * r + 65])
            nc.sync.dma_start(out=ssk_t[r:r + 1], in_=agkv_out[66 * r + 65:66 * r + 66])
        es_psSum = ExitStack()
        psSum = es_psSum.enter_context(tc.tile_pool(name="psSum", bufs=1, space="PSUM"))
        sums = [psSum.tile([1, 512], F32, tag=f"s{i}", name=f"s{i}") for i in range(4)]
        rtq_sb = mid2.tile([1, T], F32, tag="rtq_sb", name="rtq_sb")
        rtk_sb = mid2.tile([1, T], F32, tag="rtk_sb", name="rtk_sb")
        for ch in range(2):
            cs = slice(512 * ch, 512 * (ch + 1))
            nc.tensor.matmul(sums[ch], ones_bf[0:NC], ssq_t[:, cs],
                             start=True, stop=True)
            nc.tensor.matmul(sums[2 + ch], ones_bf[0:NC], ssk_t[:, cs],
                             start=True, stop=True)
            nc.scalar.activation(out=rtq_sb[:, cs], in_=sums[ch],
                                 func=mybir.ActivationFunctionType.Sqrt,
                                 bias=eps_t, scale=1.0 / RQ)
            nc.scalar.activation(out=rtk_sb[:, cs], in_=sums[2 + ch],
                                 func=mybir.ActivationFunctionType.Sqrt,
                                 bias=eps_t, scale=1.0 / RKV)
        es_psSum.close()
        invq_f = mid2.tile([1, T], F32, tag="invq_f", name="invq_f")
        nc.vector.reciprocal_approx_fast(out=invq_f, in_=rtq_sb)
        invkv_f = mid2.tile([1, T], F32, tag="invkv_f", name="invkv_f")
        nc.vector.reciprocal_approx_fast(out=invkv_f, in_=rtk_sb)
        invq_sb = mid2.tile([1, T], BF, tag="invq_sb", name="invq_sb")
        nc.vector.tensor_copy(invq_sb, invq_f)
        invkv_sb = mid2.tile([1, T], BF, tag="invkv_sb", name="invkv_sb")
        nc.vector.tensor_copy(invkv_sb, invkv_f)
        invk_ps = [psInv.tile([128, 512], F32, tag=f"ik{c}", name=f"ik{c}")
                   for c in range(2)]
        invq_ps = [psInv.tile([128, 512], F32, tag=f"iq{c}", name=f"iq{c}")
                   for c in range(2)]
        for ch in range(2):
            cs = slice(512 * ch, 512 * (ch + 1))
            nc.tensor.matmul(invk_ps[ch], onesr_f, invkv_sb[:, cs],
                             start=True, stop=True)
            nc.tensor.matmul(invq_ps[ch], onesr_f, invq_sb[:, cs],
                             start=True, stop=True)

        # k_pe rope (consumes the kpe AllReduce; off the critical path)
        kpe_sb = mid2.tile([64, T], F32, tag="kpe_sb", name="kpe_sb")
        nc.sync.dma_start(out=kpe_sb, in_=ar_out[0:64])
        kpe_sw = mid2.tile([64, T], F32, tag="kpe_sw", name="kpe_sw")
        nc.sync.dma_start(out=kpe_sw[0:32], in_=kpe_sb[32:64])
        nc.sync.dma_start(out=kpe_sw[32:64], in_=kpe_sb[0:32])
        kt1 = mid2.tile([64, T], F32, tag="kt1", name="kt1")
        kt2 = mid2.tile([64, T], F32, tag="kt2", name="kt2")
        nc.vector.tensor_mul(kt1, kpe_sb, CCb[0:64])
        nc.vector.tensor_mul(kt2, kpe_sw, SSb[0:64])
        kroped = mid2.tile([64, T], BF, tag="kroped", name="kroped")
        nc.vector.tensor_add(kroped, kt1, kt2)
        nc.sync.dma_start(out=kroped2[0:64], in_=kroped)
        nc.sync.dma_start(out=kroped2[64:128], in_=kroped)
        es_mid2.close()

        # ============ Stage B: kv_b projections ============
        ckv = [qnkv.tile([128, T], BF, tag=f"ckv{k}", name=f"ckv{k}") for k in range(KT_KV)]
        for k in range(KT_KV):
            nc.sync.dma_start(out=ckv[k][0:64],
                              in_=agkv_out[66 * 2 * k:66 * 2 * k + 64])
            nc.sync.dma_start(out=ckv[k][64:128],
                              in_=agkv_out[66 * (2 * k + 1):66 * (2 * k + 1) + 64])
            for ch in range(2):
                cs = slice(512 * ch, 512 * (ch + 1))
                nc.vector.tensor_mul(ckv[k][:, cs], ckv[k][:, cs], invk_ps[ch])

        kn = [heads.tile([128, T], BF, tag=f"kn{m}", name=f"kn{m}") for m in range(HC)]
        v_t = [heads.tile([128, HC * DV], BF, tag=f"v{t_}", name=f"v{t_}") for t_ in range(8)]

        es_s5 = ExitStack()
        s5 = es_s5.enter_context(tc.tile_pool(name="s5", bufs=3))
        es_ps5 = ExitStack()
        ps5 = es_ps5.enter_context(tc.tile_pool(name="ps5", bufs=3, space="PSUM"))
        for mt in range(HC):
            wt = s5.tile([128, KT_KV, 128], BF, tag="wkn", name="wkn")
            nc.scalar.dma_start(out=wt, in_=wkn_in[:, mt])
            pss = [ps5.tile([128, 512], F32, tag="ps", name="ps") for _ in range(2)]
            for kt in range(KT_KV):
                for ch in range(2):
                    nc.tensor.matmul(pss[ch], wt[:, kt, :],
                                     ckv[kt][:, 512 * ch:512 * (ch + 1)],
                                     start=(kt == 0), stop=(kt == KT_KV - 1))
            for ch in range(2):
                nc.scalar.copy(kn[mt][:, 512 * ch:512 * (ch + 1)], pss[ch])
        es_wvp = ExitStack()
        wvp = es_wvp.enter_context(tc.tile_pool(name="wvp", bufs=1))
        wv_t = wvp.tile([128, KT_KV, HC * DV], BF, tag="wv", name="wv")
        for kt in range(KT_KV):
            nc.scalar.dma_start(out=wv_t[:, kt], in_=wv_in[:, kt])
        for tt in range(8):
            for half in range(2):
                pss = [ps5.tile([128, 512], F32, tag="ps", name="ps") for _ in range(2)]
                for kt in range(KT_KV):
                    for c2 in range(2):
                        ch = 2 * half + c2
                        nc.tensor.matmul(
                            pss[c2], ckv[kt][:, 128 * tt:128 * (tt + 1)],
                            wv_t[:, kt, 512 * ch:512 * (ch + 1)],
                            start=(kt == 0), stop=(kt == KT_KV - 1))
                for c2 in range(2):
                    ch = 2 * half + c2
                    nc.scalar.copy(v_t[tt][:, 512 * ch:512 * (ch + 1)], pss[c2])
        es_wvp.close()
        es_ps5.close()
        es_s5.close()

        # ============ Stage C: q_b projection + RoPE ============
        qn = [qnkv.tile([128, T], BF, tag=f"qn{k}", name=f"qn{k}") for k in range(KT_Q)]
        for k in range(KT_Q):
            nc.sync.dma_start(out=qn[k], in_=agq_out[128 * k:128 * (k + 1)])
            for ch in range(2):
                cs = slice(512 * ch, 512 * (ch + 1))
                nc.vector.tensor_mul(qn[k][:, cs], qn[k][:, cs], invq_ps[ch])
        es_psInv.close()

        qh = [heads.tile([128, T], BF, tag=f"qh{m}", name=f"qh{m}") for m in range(HC)]
        qr = [heads.tile([128, T], BF, tag=f"qr{m}", name=f"qr{m}") for m in range(8)]

        es_s4 = ExitStack()
        s4 = es_s4.enter_context(tc.tile_pool(name="s4", bufs=4))
        es_s4t = ExitStack()
        s4t = es_s4t.enter_context(tc.tile_pool(name="s4t", bufs=2))
        es_ps4 = ExitStack()
        ps4 = es_ps4.enter_context(tc.tile_pool(name="ps4", bufs=4, space="PSUM"))
        # rope tiles (mt 16..23) first so attention can start right after qh
        for mt in list(range(HC, MT_QB)) + list(range(HC)):
            wt = s4.tile([128, KT_Q, 128], BF, tag="wqb", name="wqb")
            nc.scalar.dma_start(out=wt, in_=wqb_in[:, mt])
            pss = [ps4.tile([128, 512], F32, tag="ps", name="ps") for _ in range(2)]
            for kt in range(KT_Q):
                for ch in range(2):
                    nc.tensor.matmul(pss[ch], wt[:, kt, :],
                                     qn[kt][:, 512 * ch:512 * (ch + 1)],
                                     start=(kt == 0), stop=(kt == KT_Q - 1))
            if mt < HC:
                for ch in range(2):
                    nc.scalar.copy(qh[mt][:, 512 * ch:512 * (ch + 1)], pss[ch])
            else:
                rt = mt - HC
                p_bf = s4t.tile([128, T], BF, tag="p_bf", name="p_bf")
                for ch in range(2):
                    nc.scalar.copy(p_bf[:, 512 * ch:512 * (ch + 1)], pss[ch])
                p_sw = s4t.tile([128, T], BF, tag="p_sw", name="p_sw")
                nc.sync.dma_start(out=p_sw[0:32], in_=p_bf[32:64])
                nc.sync.dma_start(out=p_sw[32:64], in_=p_bf[0:32])
                nc.sync.dma_start(out=p_sw[64:96], in_=p_bf[96:128])
                nc.sync.dma_start(out=p_sw[96:128], in_=p_bf[64:96])
                t1 = s4t.tile([128, T], BF, tag="t1", name="t1")
                t2 = s4t.tile([128, T], BF, tag="t2", name="t2")
                nc.vector.tensor_mul(t1, p_bf, CCb)
                nc.vector.tensor_mul(t2, p_sw, SSb)
                nc.vector.tensor_add(qr[rt], t1, t2)
        es_ps4.close()
        es_s4t.close()
        es_s4.close()
        es_qnkv.close()

        # ============ Attention (16 heads, software-pipelined) ============
        es_s6 = ExitStack()
        s6 = es_s6.enter_context(tc.tile_pool(name="s6", bufs=5))
        es_s6a = ExitStack()
        s6a = es_s6a.enter_context(tc.tile_pool(name="s6a", bufs=4))
        es_s6b = ExitStack()
        s6b = es_s6b.enter_context(tc.tile_pool(name="s6b", bufs=2))
        es_s6d = ExitStack()
        s6d = es_s6d.enter_context(tc.tile_pool(name="s6d", bufs=3))
        es_psS = ExitStack()
        psS = es_psS.enter_context(tc.tile_pool(name="psS", bufs=3, space="PSUM"))
        es_psO = ExitStack()
        psO = es_psO.enter_context(tc.tile_pool(name="psO", bufs=3, space="PSUM"))
        es_psD = ExitStack()
        psD = es_psD.enter_context(tc.tile_pool(name="psD", bufs=2, space="PSUM"))

        # unit list: (hh, ch, jt, ns, ne) with group boundaries at (hh, ch)
        units = []
        for hh in range(HC):
            for ch in range(2):
                for jt in range(8):
                    ns, ne = max(128 * jt, 512 * ch), 512 * (ch + 1)
                    if ns < ne:
                        units.append((hh, ch, jt, ns, ne))

        gstate = {}   # (hh,ch) -> dict(pso, acc)
        inflight = deque()   # (unit, et_tile)
        gdone = deque()      # (key, enqueue_step): groups awaiting den drain
        step = [0]

        def issue(u):
            hh, ch, jt, ns, ne = u
            w = ne - ns
            rt, half = hh // 2, hh % 2
            qr_sl = qr[rt][64 * half:64 * (half + 1)]
            kp_sl = kroped2[64 * half:64 * (half + 1)]
            qlo = 128 * jt
            pst = psS.tile([128, 512], F32, tag="s", name="s")
            nc.tensor.matmul(pst[:, 0:w], kn[hh][:, qlo:qlo + 128],
                             qh[hh][:, ns:ne], start=True, stop=False)
            nc.tensor.matmul(pst[:, 0:w], kp_sl[:, qlo:qlo + 128],
                             qr_sl[:, ns:ne], start=False, stop=True)
            et = s6.tile([128, 512], BF, tag="et", name="et")
            nc.scalar.activation(out=et[:, 0:w], in_=pst[:, 0:w],
                                 func=mybir.ActivationFunctionType.Exp,
                                 scale=SCALE)
            if ns == qlo:
                nc.vector.tensor_mul(et[:, 0:128], et[:, 0:128], mask_t)
            inflight.append((u, et))

        def drain_group(force=False):
            if not gdone:
                return
            if not force and gdone[0][1] >= step[0]:
                return
            (hh, ch), _ = gdone.popleft()
            g = gstate.pop((hh, ch))
            cs = slice(512 * ch, 512 * (ch + 1))
            accb = s6.tile([128, 512], BF, tag="accb", name="accb")
            nc.vector.tensor_copy(accb, g["acc"])
            dbc = psD.tile([128, 512], F32, tag="d", name="d")
            nc.tensor.matmul(dbc, ones128, accb, start=True, stop=True)
            rec_bc = s6b.tile([128, 512], F32, tag="rbc", name="rbc")
            nc.vector.reciprocal_approx_fast(out=rec_bc, in_=dbc)
            obf = s6d.tile([128, 512], BF, tag="obf", name="obf")
            nc.vector.tensor_mul(obf, g["pso"], rec_bc)
            nc.sync.dma_start(
                out=ago_in[hh // 4][128 * (hh % 4):128 * (hh % 4 + 1), cs],
                in_=obf)
            if ch == 1 and hh % 4 == 3:
                i = hh // 4
                nc.gpsimd.collective_compute(
                    "AllGather", mybir.AluOpType.bypass, replica_groups=RG,
                    ins=[ago_in[i].opt()], outs=[ago_out[i].opt()])

        def finalize():
            u, et = inflight.popleft()
            hh, ch, jt, ns, ne = u
            w = ne - ns
            key = (hh, ch)
            first = key not in gstate
            if first:
                gstate[key] = dict(
                    pso=psO.tile([128, 512], F32, tag="o", name="o"),
                    acc=s6a.tile([128, 512], F32, tag="acc", name="acc"))
            g = gstate[key]
            os_ = ns - 512 * ch
            last = (jt == 3) if ch == 0 else (jt == 7)
            nc.tensor.matmul(g["pso"][:, os_:512],
                             v_t[jt][:, 128 * hh:128 * (hh + 1)],
                             et[:, 0:w], start=first, stop=last,
                             skip_group_check=True)
            if first:
                nc.vector.tensor_copy(g["acc"], et[:, 0:w])
            else:
                nc.vector.tensor_add(g["acc"][:, os_:512], g["acc"][:, os_:512],
                                     et[:, 0:w])
            if last:
                gdone.append((key, step[0]))

        LOOKAHEAD = 2
        for idx, u in enumerate(units):
            issue(u)
            if idx >= LOOKAHEAD:
                finalize()
            drain_group()
            step[0] += 1
        while inflight:
            finalize()
            drain_group()
            step[0] += 1
        while gdone:
            drain_group(force=True)

        es_psD.close()
        es_psO.close()
        es_psS.close()
        es_s6d.close()
        es_s6b.close()
        es_s6a.close()
        es_s6.close()
        es_heads.close()

        # ============ o_proj (row shard, K = 16384) ============
        es_s7w = ExitStack()
        s7w = es_s7w.enter_context(tc.tile_pool(name="s7w", bufs=6))
        es_s7r = ExitStack()
        s7r = es_s7r.enter_context(tc.tile_pool(name="s7r", bufs=6))
        es_s7o = ExitStack()
        s7o = es_s7o.enter_context(tc.tile_pool(name="s7o", bufs=4))
        es_ps7 = ExitStack()
        ps7 = es_ps7.enter_context(tc.tile_pool(name="ps7", bufs=1, space="PSUM"))
        for ch in range(2):
            pso7 = [ps7.tile([128, 512], F32, tag=f"m{m}", name=f"m{m}")
                    for m in range(7)]
            for i in range(4):
                for c in range(NC):
                    for j in range(4):
                        ktg = 16 * c + 4 * i + j
                        wt = s7w.tile([128, 7, 128], BF, tag="wo", name="wo")
                        nc.scalar.dma_start(out=wt, in_=wo_in[:, ktg])
                        rh = s7r.tile([128, 512], BF, tag="rh", name="rh")
                        nc.sync.dma_start(
                            out=rh,
                            in_=ago_out[i][512 * c + 128 * j:512 * c + 128 * (j + 1),
                                           512 * ch:512 * (ch + 1)])
                        st = (i == 0 and c == 0 and j == 0)
                        sp = (i == 3 and c == NC - 1 and j == 3)
                        for mtt in range(7):
                            nc.tensor.matmul(pso7[mtt], wt[:, mtt, :], rh,
                                             start=st, stop=sp)
            for mtt in range(7):
                ot = s7o.tile([128, 512], F32, tag="ot", name="ot")
                nc.scalar.copy(ot, pso7[mtt])
                nc.sync.dma_start(
                    out=out_ap[128 * mtt:128 * (mtt + 1), 512 * ch:512 * (ch + 1)],
                    in_=ot)
        es_ps7.close()
        es_s7o.close()
        es_s7r.close()
        es_s7w.close()

        es_persist.close()
        es_dram.close()

    nc.finalize()
    return nc
